# revision 33
# baseline (speedup 1.0000x reference)
"""Trainium2 kernel for GaPartNet NMS detection.

Pipeline:
  1. (host)   scatter 1M (proposal, point) pairs into dense membership
              matrix X[point, proposal] = count   [100352, 512] (padded)
  2. (device) 8-way shard over the point dim: core d computes the upper
              block-triangle of X_d.T @ X_d  ([512,512] partial
              intersection) with fp8 DoubleRow matmuls (counts are small
              integers -> exact in fp8-e4m3 with fp32 PSUM accumulation)
  3. (host)   sum the 8 partials, mirror the triangle, IoU, greedy NMS
"""

import numpy as np
import ml_dtypes

P = 512            # proposals
NPOINT = 100000    # points (CSR column space)
NCORES = 8
ROWS_PER = 12500   # real point-rows per core
KT = 98            # k-tiles of 128 rows per core (12544 padded rows)
NPAD = KT * 128    # 12544
# Input DMA slabs (k-tiles each, all even so DoubleRow pairs stay in-slab).
# Measured on this platform: ~9us fixed DMA startup, then the HWDGE ring
# streams queued DMAs in order at ~350 GB/s. So: queue many small slabs
# unchained — the first slab lands early (PE starts ~11.5us) and the stream
# stays ahead of PE (0.19us/tile DMA vs 0.25+us/tile PE).
# ramped: tiny first slab so PE starts ASAP after the fixed DMA startup
SLABS = [2, 4, 6, 8, 10, 12, 12, 12, 12, 12, 8]
NPAIR = KT // 2    # 49 DoubleRow k-tile pairs
IOU_THRESHOLD = 0.3
MBLK = 4           # 512 = 4 x 128 output row-blocks
OFFS = [0, 512, 896, 1152]   # packed col offset of each row-block's output
PACKW = 1280       # 512 + 384 + 256 + 128

_CACHE = {}
LAST_RUN = None    # BassKernelResults of the most recent device run


def _build_nc(mode):
    """mode: 'fp8dr' (fp8 + DoubleRow) or 'bf16' (plain matmul).

    Raw bass (no TileContext): this toolchain's walrus codegen rejects any
    instruction carrying more than ~2 sync waits, and Tile's auto-emitted
    tail drain always accumulates one wait per used processor. With explicit
    blocks every wait is single-semaphore."""
    import concourse.bass as bass
    import concourse.mybir as mybir

    F32 = mybir.dt.float32
    DT = mybir.dt.float8e4 if mode == "fp8dr" else mybir.dt.bfloat16

    nc = bass.Bass()
    x = nc.declare_dram_parameter("x", [128, KT, P], DT, isOutput=False)
    # packed upper block-triangle: m-th block is [128, 512-128m] at col OFFS[m]
    out = nc.declare_dram_parameter("out", [128, PACKW], F32, isOutput=True)

    import contextlib

    with contextlib.ExitStack() as ctx:
        # one sem per input slab: CoreSim's race detector rejects concurrent
        # in-flight DMAs updating a shared semaphore
        s_in = [
            ctx.enter_context(nc.semaphore(f"s_in{i}")) for i in range(len(SLABS))
        ]
        s_mm = ctx.enter_context(nc.semaphore("s_mm"))
        s_cp = ctx.enter_context(nc.semaphore("s_cp"))
        s_out = [
            ctx.enter_context(nc.semaphore(f"s_out{m}")) for m in range(MBLK)
        ]
        xt = ctx.enter_context(nc.sbuf_tensor("xt", [128, KT, P], DT))
        acc = [
            ctx.enter_context(
                nc.psum_tensor(f"acc{m}", [128, P - 128 * m], F32)
            )
            for m in range(MBLK)
        ]
        res = ctx.enter_context(nc.sbuf_tensor("res", [128, PACKW], F32))
        # HAM warm-up resources: PE runs dummy matmuls on a memset tile into a
        # junk PSUM bank during the ~9us DMA-startup window, so the real
        # stream runs at 2.4GHz from its first instruction
        s_wm = ctx.enter_context(nc.semaphore("s_wm"))
        s_wu = ctx.enter_context(nc.semaphore("s_wu"))
        warm = ctx.enter_context(nc.sbuf_tensor("warm", [128, 2, P], DT))
        junk = ctx.enter_context(nc.psum_tensor("junk", [128, P], F32))
        with nc.Block() as block:

            @block.sync
            def _(sync):
                # unchained: the HWDGE ring executes queued DMAs in order, so
                # slab 0 completes first; no sequencer round-trips needed
                t0 = 0
                for si, tps in enumerate(SLABS):
                    sync.dma_start(
                        out=xt[:, t0 : t0 + tps, :], in_=x[:, t0 : t0 + tps, :]
                    ).then_inc(s_in[si], 16)
                    t0 += tps
                for m in range(MBLK):
                    sync.wait_ge(s_out[m], 16)

            @block.scalar
            def _(sc):
                # tiny early DMA pays the qAct ring's first-use latency now;
                # its junk bytes land in a region the real m=0 DMA overwrites
                sc.wait_ge(s_wm, 2)
                sc.dma_start(out=out[0:1, 0:4], in_=res[0:1, 0:4]).then_inc(
                    s_wu, 16
                )
                sc.wait_ge(s_wu, 16)
                # output DMAs on the idle qAct ring so they never queue behind
                # the input stream; each block ships right after its copy
                for m in range(MBLK):
                    sc.wait_ge(s_cp, m + 1)
                    sc.dma_start(
                        out=out[:, OFFS[m] : OFFS[m] + (P - 128 * m)],
                        in_=res[:, OFFS[m] : OFFS[m] + (P - 128 * m)],
                    ).then_inc(s_out[m], 16)

            @block.gpsimd
            def _(gp):
                gp.memset(warm[:, :, :], 0).then_inc(s_wm, 1)
                gp.memset(res[0:1, 0:4], 0).then_inc(s_wm, 1)

            @block.tensor
            def _(pe):
                # ~25 dummy matmuls (~6us) warm the HAM clock gate to 2.4GHz
                # while the input DMA path starts up; junk bank is never read
                pe.wait_ge(s_wm, 1)
                for _ in range(25):
                    if mode == "fp8dr":
                        pe.matmul(
                            junk[:, :], warm[:, :, 0:128], warm[:, :, :],
                            start=True, stop=True,
                            perf_mode=mybir.MatmulPerfMode.DoubleRow,
                        )
                    else:
                        pe.matmul(
                            junk[:, :], warm[:, 0, 0:128], warm[:, 0, :],
                            start=True, stop=True,
                        )
                step = 0
                nsteps = NPAIR if mode == "fp8dr" else KT
                t0 = 0
                for si, tps in enumerate(SLABS):
                    pe.wait_ge(s_in[si], 16)
                    if mode == "fp8dr":
                        for j in range(tps // 2):
                            t = t0 + 2 * j
                            for m in range(MBLK):
                                mm = pe.matmul(
                                    acc[m][:],
                                    xt[:, t : t + 2, 128 * m : 128 * (m + 1)],
                                    xt[:, t : t + 2, 128 * m :],
                                    start=(step == 0), stop=(step == nsteps - 1),
                                    perf_mode=mybir.MatmulPerfMode.DoubleRow,
                                )
                                if step == nsteps - 1:
                                    mm.then_inc(s_mm, 1)
                            step += 1
                    else:
                        for j in range(tps):
                            t = t0 + j
                            for m in range(MBLK):
                                mm = pe.matmul(
                                    acc[m][:],
                                    xt[:, t, 128 * m : 128 * (m + 1)],
                                    xt[:, t, 128 * m :],
                                    start=(step == 0), stop=(step == nsteps - 1),
                                )
                                if step == nsteps - 1:
                                    mm.then_inc(s_mm, 1)
                            step += 1
                    t0 += tps

            @block.vector
            def _(vec):
                # order after the warm-up DMA's read of res (WAR, sim-only)
                vec.wait_ge(s_wu, 16)
                # bank m's last matmul incs s_mm; copy each bank as it lands
                for m in range(MBLK):
                    vec.wait_ge(s_mm, m + 1)
                    vec.tensor_copy(
                        res[:, OFFS[m] : OFFS[m] + (P - 128 * m)], acc[m][:]
                    ).then_inc(s_cp, 1)

    return nc


def _get_nc(mode):
    if mode not in _CACHE:
        _CACHE[mode] = _build_nc(mode)
    return _CACHE[mode]


def _greedy_nms_host(ious, scores, threshold):
    p = scores.shape[0]
    order = np.argsort(-scores, kind="stable")
    iou_s = ious[order][:, order]
    keep_sorted = np.ones(p, dtype=bool)
    idx = np.arange(p)
    for i in range(p):
        if keep_sorted[i]:
            keep_sorted &= ~((idx > i) & (iou_s[i] > threshold))
    keep = np.empty(p, dtype=bool)
    keep[order] = keep_sorted
    return keep


def _ensure_ntff_hook():
    """This image's `antenv` lacks `axon_hooks`, which bass_utils imports
    unconditionally when tracing under axon. Provide a compatible shim backed
    by trn_agent_boot's ctypes NTFF driver (returns None -> tracing is
    skipped gracefully if that isn't available either)."""
    import sys
    import types

    try:
        import antenv.axon_hooks  # noqa: F401
        return
    except ImportError:
        pass
    mod = types.ModuleType("antenv.axon_hooks")
    mod._hook = None

    def set_axon_ntff_profile_hook(h):
        mod._hook = h

    def get_axon_ntff_profile_hook():
        if mod._hook is None:
            try:
                from trn_agent_boot.trn_boot import _ntff_profile_via_ctypes

                mod._hook = _ntff_profile_via_ctypes("/opt/axon/libaxon_pjrt.so")
            except Exception:
                return None
        return mod._hook

    mod.set_axon_ntff_profile_hook = set_axon_ntff_profile_hook
    mod.get_axon_ntff_profile_hook = get_axon_ntff_profile_hook
    sys.modules["antenv.axon_hooks"] = mod
    try:
        import antenv

        antenv.axon_hooks = mod
    except ImportError:
        pass


def kernel(sorted_indices, proposal_indices, values, score_preds, num_points):
    global LAST_RUN
    _ensure_ntff_hook()
    from concourse.bass_utils import run_bass_kernel_spmd

    si = np.asarray(sorted_indices).astype(np.int64, copy=False)
    pi = np.asarray(proposal_indices).astype(np.int64, copy=False)
    vals = np.asarray(values).astype(np.float64, copy=False)
    scores = np.asarray(score_preds).astype(np.float32, copy=False)
    n_points = int(num_points)
    assert n_points == NPOINT and scores.shape[0] == P

    # --- host: build dense membership matrix, point-major ---
    flat = si * P + pi
    cnt = np.bincount(flat, weights=vals, minlength=NPOINT * P)
    cnt = cnt.reshape(NPOINT, P)
    cmax = cnt.max()
    mode = "fp8dr" if cmax <= 15 else ("bf16" if cmax <= 256 else None)
    in_dt = ml_dtypes.float8_e4m3 if mode == "fp8dr" else ml_dtypes.bfloat16
    assert mode is not None, f"counts too large for exact low-precision: {cmax}"

    # per-proposal point totals (row sums of A), exact integers
    n_per = np.bincount(pi, weights=vals, minlength=P).astype(np.float32)

    in_maps = []
    for d in range(NCORES):
        shard = np.zeros((NPAD, P), dtype=in_dt)
        shard[:ROWS_PER] = cnt[d * ROWS_PER : (d + 1) * ROWS_PER]
        # [KT*128, P] -> [128, KT, P] so each SBUF partition reads contiguously
        shard = np.ascontiguousarray(shard.reshape(KT, 128, P).transpose(1, 0, 2))
        in_maps.append({"x": shard})

    # --- device: sharded partial intersections ---
    nc = _get_nc(mode)
    res = run_bass_kernel_spmd(nc, in_maps, list(range(NCORES)))
    LAST_RUN = res

    # --- host: reduce partials, unpack + mirror triangle, IoU, NMS ---
    packed = np.zeros((128, PACKW), dtype=np.float64)
    for d in range(NCORES):
        packed += res.results[d]["out"].astype(np.float64)
    inter = np.zeros((P, P), dtype=np.float64)
    for m in range(MBLK):
        w = P - 128 * m
        inter[128 * m : 128 * (m + 1), 128 * m :] = packed[:, OFFS[m] : OFFS[m] + w]
    inter = np.triu(inter) + np.triu(inter, 1).T
    inter = inter.astype(np.float32)

    union = n_per[:, None] + n_per[None, :] - inter
    ious = inter / (union + np.float32(1e-8))
    keep = _greedy_nms_host(ious, scores, IOU_THRESHOLD)
    return ious.astype(np.float32), keep


# revision 34
# speedup vs baseline: 1.0646x; 1.0646x over previous
"""Trainium2 kernel for GaPartNet NMS detection.

Pipeline:
  1. (host)   scatter 1M (proposal, point) pairs into dense membership
              matrix X[point, proposal] = count   [100352, 512] (padded)
  2. (device) 8-way shard over the point dim: core d computes the upper
              block-triangle of X_d.T @ X_d  ([512,512] partial
              intersection) with fp8 DoubleRow matmuls (counts are small
              integers -> exact in fp8-e4m3 with fp32 PSUM accumulation)
  3. (host)   sum the 8 partials, mirror the triangle, IoU, greedy NMS
"""

import numpy as np
import ml_dtypes

P = 512            # proposals
NPOINT = 100000    # points (CSR column space)
NCORES = 8
ROWS_PER = 12500   # real point-rows per core
KT = 98            # k-tiles of 128 rows per core (12544 padded rows)
NPAD = KT * 128    # 12544
# Input DMA slabs (k-tiles each, all even so DoubleRow pairs stay in-slab).
# Measured on this platform: ~9us fixed DMA startup, then the HWDGE ring
# streams queued DMAs in order at ~350 GB/s. So: queue many small slabs
# unchained — the first slab lands early (PE starts ~11.5us) and the stream
# stays ahead of PE (0.19us/tile DMA vs 0.25+us/tile PE).
# ramped: tiny first slab so PE starts ASAP after the fixed DMA startup
SLABS = [2, 4, 6, 8, 10, 12, 12, 12, 12, 12, 8]
NPAIR = KT // 2    # 49 DoubleRow k-tile pairs
IOU_THRESHOLD = 0.3
MBLK = 4           # 512 = 4 x 128 output row-blocks
OFFS = [0, 512, 896, 1152]   # packed col offset of each row-block's output
PACKW = 1280       # 512 + 384 + 256 + 128

_CACHE = {}
LAST_RUN = None    # BassKernelResults of the most recent device run


def _build_nc(mode):
    """mode: 'fp8dr' (fp8 + DoubleRow) or 'bf16' (plain matmul).

    Raw bass (no TileContext): this toolchain's walrus codegen rejects any
    instruction carrying more than ~2 sync waits, and Tile's auto-emitted
    tail drain always accumulates one wait per used processor. With explicit
    blocks every wait is single-semaphore."""
    import concourse.bass as bass
    import concourse.mybir as mybir

    F32 = mybir.dt.float32
    DT = mybir.dt.float8e4 if mode == "fp8dr" else mybir.dt.bfloat16

    nc = bass.Bass()
    x = nc.declare_dram_parameter("x", [128, KT, P], DT, isOutput=False)
    # packed upper block-triangle: m-th block is [128, 512-128m] at col OFFS[m]
    out = nc.declare_dram_parameter("out", [128, PACKW], F32, isOutput=True)

    import contextlib

    with contextlib.ExitStack() as ctx:
        # one sem per input slab: CoreSim's race detector rejects concurrent
        # in-flight DMAs updating a shared semaphore
        s_in = [
            ctx.enter_context(nc.semaphore(f"s_in{i}")) for i in range(len(SLABS))
        ]
        s_mm = ctx.enter_context(nc.semaphore("s_mm"))
        s_cp = ctx.enter_context(nc.semaphore("s_cp"))
        s_out = [
            ctx.enter_context(nc.semaphore(f"s_out{m}")) for m in range(MBLK)
        ]
        xt = ctx.enter_context(nc.sbuf_tensor("xt", [128, KT, P], DT))
        acc = [
            ctx.enter_context(
                nc.psum_tensor(f"acc{m}", [128, P - 128 * m], F32)
            )
            for m in range(MBLK)
        ]
        res = ctx.enter_context(nc.sbuf_tensor("res", [128, PACKW], F32))
        with nc.Block() as block:

            @block.sync
            def _(sync):
                # unchained: the HWDGE ring executes queued DMAs in order, so
                # slab 0 completes first; no sequencer round-trips needed
                t0 = 0
                for si, tps in enumerate(SLABS):
                    sync.dma_start(
                        out=xt[:, t0 : t0 + tps, :], in_=x[:, t0 : t0 + tps, :]
                    ).then_inc(s_in[si], 16)
                    t0 += tps
                for m in range(MBLK):
                    sync.wait_ge(s_out[m], 16)

            @block.scalar
            def _(sc):
                # output DMAs on the idle qAct ring so they never queue behind
                # the input stream; each block ships right after its copy
                for m in range(MBLK):
                    sc.wait_ge(s_cp, m + 1)
                    sc.dma_start(
                        out=out[:, OFFS[m] : OFFS[m] + (P - 128 * m)],
                        in_=res[:, OFFS[m] : OFFS[m] + (P - 128 * m)],
                    ).then_inc(s_out[m], 16)

            @block.tensor
            def _(pe):
                step = 0
                nsteps = NPAIR if mode == "fp8dr" else KT
                t0 = 0
                for si, tps in enumerate(SLABS):
                    pe.wait_ge(s_in[si], 16)
                    if mode == "fp8dr":
                        for j in range(tps // 2):
                            t = t0 + 2 * j
                            for m in range(MBLK):
                                mm = pe.matmul(
                                    acc[m][:],
                                    xt[:, t : t + 2, 128 * m : 128 * (m + 1)],
                                    xt[:, t : t + 2, 128 * m :],
                                    start=(step == 0), stop=(step == nsteps - 1),
                                    perf_mode=mybir.MatmulPerfMode.DoubleRow,
                                )
                                if step == nsteps - 1:
                                    mm.then_inc(s_mm, 1)
                            step += 1
                    else:
                        for j in range(tps):
                            t = t0 + j
                            for m in range(MBLK):
                                mm = pe.matmul(
                                    acc[m][:],
                                    xt[:, t, 128 * m : 128 * (m + 1)],
                                    xt[:, t, 128 * m :],
                                    start=(step == 0), stop=(step == nsteps - 1),
                                )
                                if step == nsteps - 1:
                                    mm.then_inc(s_mm, 1)
                            step += 1
                    t0 += tps

            @block.vector
            def _(vec):
                # bank m's last matmul incs s_mm; copy each bank as it lands
                for m in range(MBLK):
                    vec.wait_ge(s_mm, m + 1)
                    vec.tensor_copy(
                        res[:, OFFS[m] : OFFS[m] + (P - 128 * m)], acc[m][:]
                    ).then_inc(s_cp, 1)

    return nc


def _get_nc(mode):
    if mode not in _CACHE:
        _CACHE[mode] = _build_nc(mode)
    return _CACHE[mode]


def _greedy_nms_host(ious, scores, threshold):
    p = scores.shape[0]
    order = np.argsort(-scores, kind="stable")
    iou_s = ious[order][:, order]
    keep_sorted = np.ones(p, dtype=bool)
    idx = np.arange(p)
    for i in range(p):
        if keep_sorted[i]:
            keep_sorted &= ~((idx > i) & (iou_s[i] > threshold))
    keep = np.empty(p, dtype=bool)
    keep[order] = keep_sorted
    return keep


def _ensure_ntff_hook():
    """This image's `antenv` lacks `axon_hooks`, which bass_utils imports
    unconditionally when tracing under axon. Provide a compatible shim backed
    by trn_agent_boot's ctypes NTFF driver (returns None -> tracing is
    skipped gracefully if that isn't available either)."""
    import sys
    import types

    try:
        import antenv.axon_hooks  # noqa: F401
        return
    except ImportError:
        pass
    mod = types.ModuleType("antenv.axon_hooks")
    mod._hook = None

    def set_axon_ntff_profile_hook(h):
        mod._hook = h

    def get_axon_ntff_profile_hook():
        if mod._hook is None:
            try:
                from trn_agent_boot.trn_boot import _ntff_profile_via_ctypes

                mod._hook = _ntff_profile_via_ctypes("/opt/axon/libaxon_pjrt.so")
            except Exception:
                return None
        return mod._hook

    mod.set_axon_ntff_profile_hook = set_axon_ntff_profile_hook
    mod.get_axon_ntff_profile_hook = get_axon_ntff_profile_hook
    sys.modules["antenv.axon_hooks"] = mod
    try:
        import antenv

        antenv.axon_hooks = mod
    except ImportError:
        pass


def kernel(sorted_indices, proposal_indices, values, score_preds, num_points):
    global LAST_RUN
    _ensure_ntff_hook()
    from concourse.bass_utils import run_bass_kernel_spmd

    si = np.asarray(sorted_indices).astype(np.int64, copy=False)
    pi = np.asarray(proposal_indices).astype(np.int64, copy=False)
    vals = np.asarray(values).astype(np.float64, copy=False)
    scores = np.asarray(score_preds).astype(np.float32, copy=False)
    n_points = int(num_points)
    assert n_points == NPOINT and scores.shape[0] == P

    # --- host: build dense membership matrix, point-major ---
    flat = si * P + pi
    cnt = np.bincount(flat, weights=vals, minlength=NPOINT * P)
    cnt = cnt.reshape(NPOINT, P)
    cmax = cnt.max()
    mode = "fp8dr" if cmax <= 15 else ("bf16" if cmax <= 256 else None)
    in_dt = ml_dtypes.float8_e4m3 if mode == "fp8dr" else ml_dtypes.bfloat16
    assert mode is not None, f"counts too large for exact low-precision: {cmax}"

    # per-proposal point totals (row sums of A), exact integers
    n_per = np.bincount(pi, weights=vals, minlength=P).astype(np.float32)

    in_maps = []
    for d in range(NCORES):
        shard = np.zeros((NPAD, P), dtype=in_dt)
        shard[:ROWS_PER] = cnt[d * ROWS_PER : (d + 1) * ROWS_PER]
        # [KT*128, P] -> [128, KT, P] so each SBUF partition reads contiguously
        shard = np.ascontiguousarray(shard.reshape(KT, 128, P).transpose(1, 0, 2))
        in_maps.append({"x": shard})

    # --- device: sharded partial intersections ---
    nc = _get_nc(mode)
    res = run_bass_kernel_spmd(nc, in_maps, list(range(NCORES)))
    LAST_RUN = res

    # --- host: reduce partials, unpack + mirror triangle, IoU, NMS ---
    packed = np.zeros((128, PACKW), dtype=np.float64)
    for d in range(NCORES):
        packed += res.results[d]["out"].astype(np.float64)
    inter = np.zeros((P, P), dtype=np.float64)
    for m in range(MBLK):
        w = P - 128 * m
        inter[128 * m : 128 * (m + 1), 128 * m :] = packed[:, OFFS[m] : OFFS[m] + w]
    inter = np.triu(inter) + np.triu(inter, 1).T
    inter = inter.astype(np.float32)

    union = n_per[:, None] + n_per[None, :] - inter
    ious = inter / (union + np.float32(1e-8))
    keep = _greedy_nms_host(ious, scores, IOU_THRESHOLD)
    return ious.astype(np.float32), keep
